# revision 6
# baseline (speedup 1.0000x reference)
"""Trainium2 Bass kernel: Minkowski-style sparse-conv BasicBlock.

  out = relu(bn2(conv2(relu(bn1(conv1(x))))) + x)

conv(x)[i] = sum_k mask[i,k] * x[nbr[i,k]] @ W[k]   (K=27 offsets, C=32 ch)
BN is training-mode (batch statistics over all N=1e6 rows).

Sharding: data-parallel over points across 8 NeuronCores; gather tables
replicated (x) / AllGather'd (raw h1).

Key design points vs a naive implementation:
 - ONE batched indirect DMA per 512-row supertile (14336 indices) instead
   of one per (k, j-block): amortizes the ~1us SWDGE per-call fixed cost.
 - fp16 tables + fp16 GEMM path (fp32 PSUM accumulate): halves gather
   bytes and runs the PE at full rate.
 - masked edges are skipped entirely via bounds_check (index sentinel
   0x7FFFFF00 > bounds): ~50% fewer gather descriptors. G tiles are
   pre-filled with 0 (conv1) / -60000 (conv2) so skipped slots contribute
   nothing after the (monotone) BN affine + relu.
 - conv1 has NO second pass: raw (pre-BN) h1 is written row-major and
   AllGather'd during pass A1; BN1+ReLU is folded into conv2 as a
   per-partition affine+relu on the transposed slabs (channel lives on
   partitions there), fused with the PSUM->SBUF copy on the ACT engine.
   Requires sign(gamma1) uniform (gamma1=1 here): sentinel slots then
   relu to exactly 0.

Per-core pass A supertile pipeline (512 rows):
  idx load [128,112] -> fill G -> 1 indirect gather (skip-masked) ->
  28 PE transposes -> (conv2: fused affine+relu) -> 7 fp16 matmuls ->
  ht[32,512] PSUM -> ACT copy (+sum/sumsq accum for BN) ->
  conv1: transpose-back to rows -> h1loc; conv2: -> scr2 channel-major.
BN stats AllReduce'd ([32,2]); pass B2 applies bn2 + residual + relu.
"""

import numpy as np
from contextlib import ExitStack

import concourse.bass as bass
import concourse.tile as tile
import concourse.mybir as mybir
from concourse import bacc
from concourse.bass import IndirectOffsetOnAxis
from concourse.masks import make_identity
from concourse.bass_utils import run_bass_kernel_spmd

F32 = mybir.dt.float32
F16 = mybir.dt.float16
I32 = mybir.dt.int32
AF = mybir.ActivationFunctionType
ALU = mybir.AluOpType

N_FULL = 1_000_000
C = 32
K = 27
KP = 28            # k padded to 28 => 28*32 = 896 = 7 slabs of 128
EPS = 1e-5
N_CORES = 8
SUP = 512          # rows per supertile
ROWS_PC = N_FULL // N_CORES            # 125000
T_FULL = -(-ROWS_PC // SUP)            # 245
R_FULL = T_FULL * SUP                  # 125440
OOB = 0x7FFFFF00   # skip sentinel (> any bounds_check value)
NEG = -60000.0     # conv2 G fill; relu(a*NEG+b) == 0 for a > ~1e-3


def build_program(R, T, n_true, n_cores=N_CORES, debug_taps=False):
    """Build the SPMD Bass program (identical on every core)."""
    assert R == T * SUP
    tbl2 = n_cores * R
    nc = bacc.Bacc(None, num_devices=n_cores)

    xt16 = nc.declare_dram_parameter("xt16", [N_FULL + 1, C], F16, isOutput=False)
    xres = nc.declare_dram_parameter("xres", [R, C], F32, isOutput=False)
    idx1 = nc.declare_dram_parameter("idx1", [T, 128, 4 * KP], I32, isOutput=False)
    idx2 = nc.declare_dram_parameter("idx2", [T, 128, 4 * KP], I32, isOutput=False)
    w1c = nc.declare_dram_parameter("w1cat", [KP * C, C], F16, isOutput=False)
    w2c = nc.declare_dram_parameter("w2cat", [KP * C, C], F16, isOutput=False)
    gb1 = nc.declare_dram_parameter("gb1", [2, C], F32, isOutput=False)
    gb2 = nc.declare_dram_parameter("gb2", [2, C], F32, isOutput=False)
    outd = nc.declare_dram_parameter("out", [R, C], F32, isOutput=True)

    scr2 = nc.dram_tensor("scr2", [C, R], F16)
    h1loc = nc.dram_tensor("h1loc", [R, C], F16)
    h1ag = nc.dram_tensor("h1ag", [tbl2 + 1, C], F16, addr_space="Shared")
    st1i = nc.dram_tensor("st1i", [C, 2], F32)
    st1o = nc.dram_tensor("st1o", [C, 2], F32, addr_space="Shared")
    st2i = nc.dram_tensor("st2i", [C, 2], F32)
    st2o = nc.dram_tensor("st2o", [C, 2], F32, addr_space="Shared")
    ab1d = nc.dram_tensor("ab1d", [C, 2], F32)

    groups = [list(range(n_cores))]
    NI = 4 * KP * 128          # indices per supertile gather

    with ExitStack() as ctx:
        tc = ctx.enter_context(tile.TileContext(nc))
        cpool = ctx.enter_context(tc.tile_pool(name="const", bufs=1))
        id16 = cpool.tile([128, 128], F16, tag="id16")
        make_identity(nc, id16[:])
        id32 = cpool.tile([C, C], F32, tag="id32")
        make_identity(nc, id32[:])

        def load_w(wc, nm):
            ws = []
            for s in range(7):
                wt = cpool.tile([128, C], F16, tag=f"{nm}_{s}")
                nc.sync.dma_start(out=wt[:], in_=wc[s * 128:(s + 1) * 128, :])
                ws.append(wt)
            return ws

        w1s = load_w(w1c, "w1")
        w2s = load_w(w2c, "w2")

        gb1t = cpool.tile([C, 2], F32, tag="gb1")
        nc.sync.dma_start(out=gb1t[:], in_=gb1[:].rearrange("a c -> c a"))
        gb2t = cpool.tile([C, 2], F32, tag="gb2")
        nc.sync.dma_start(out=gb2t[:], in_=gb2[:].rearrange("a c -> c a"))

        negrow = cpool.tile([1, C], F16, tag="negrow")
        nc.vector.memset(negrow[:], NEG)
        nc.gpsimd.dma_start(out=h1ag[tbl2:tbl2 + 1, :], in_=negrow[:])

        stat = ctx.enter_context(tc.tile_pool(name="stat", bufs=1))
        sacc1 = stat.tile([C, 2], F32, tag="sacc1")
        sacc2 = stat.tile([C, 2], F32, tag="sacc2")
        ab1 = stat.tile([C, 2], F32, tag="ab1")
        ab2 = stat.tile([C, 2], F32, tag="ab2")
        ab1f = stat.tile([128, 2], F32, tag="ab1f")   # ab1 tiled 4x on partitions

        pidx = ctx.enter_context(tc.tile_pool(name="pidx", bufs=3))
        pg = ctx.enter_context(tc.tile_pool(name="pg", bufs=3))
        pgt = ctx.enter_context(tc.tile_pool(name="pgt", bufs=2))
        ppt = ctx.enter_context(tc.tile_pool(name="ppt", bufs=3, space="PSUM"))
        pht = ctx.enter_context(tc.tile_pool(name="pht", bufs=2, space="PSUM"))
        prow = ctx.enter_context(tc.tile_pool(name="prow", bufs=1, space="PSUM"))
        phsb = ctx.enter_context(tc.tile_pool(name="phsb", bufs=3))
        psq = ctx.enter_context(tc.tile_pool(name="psq", bufs=2))
        psc = ctx.enter_context(tc.tile_pool(name="psc", bufs=1))

        def conv_passA(idxd, table, fill, ws, affine, ssum, emit_tail):
            """One supertile pass: gather -> transpose -> [affine] -> GEMM.

            emit_tail(t, ht) consumes the [C, SUP] fp32 PSUM result."""
            nc.vector.memset(ssum[:], 0.0)

            def body(t):
                idxt = pidx.tile([128, 4 * KP], I32, tag="idx")
                nc.sync.dma_start(out=idxt[:],
                                  in_=idxd[bass.ds(t, 1), :, :].squeeze(0))
                G = pg.tile([128, 4 * KP, C], F16, tag="g")
                for j in range(4):
                    nc.vector.memset(G[:, j * KP + K, :], fill)  # pad col
                    for k in range(K):
                        c = j * KP + k
                        nc.gpsimd.indirect_dma_start(
                            out=G[:, c, :], out_offset=None,
                            in_=table[:],
                            in_offset=IndirectOffsetOnAxis(
                                ap=idxt[:, c:c + 1], axis=0),
                        )
                Gf = G[:].rearrange("p a c -> p (a c)")
                GT = pgt.tile([128, 7, SUP], F16, tag="gt")
                for s in range(7):
                    pt = ppt.tile([128, SUP], F16, tag="pt")
                    for j in range(4):
                        nc.tensor.transpose(
                            out=pt[:, j * 128:(j + 1) * 128],
                            in_=Gf[:, j * KP * C + s * 128:
                                   j * KP * C + s * 128 + 128],
                            identity=id16[:],
                        )
                    if affine is not None:
                        # fused PSUM->SBUF copy + bn1 affine + relu
                        nc.scalar.activation(
                            out=GT[:, s, :], in_=pt[:], func=AF.Relu,
                            scale=affine[:, 0:1], bias=affine[:, 1:2])
                    elif s % 2 == 0:
                        nc.scalar.copy(out=GT[:, s, :], in_=pt[:])
                    else:
                        nc.vector.tensor_copy(out=GT[:, s, :], in_=pt[:])
                ht = pht.tile([C, SUP], F32, tag="ht")
                for s in range(7):
                    nc.tensor.matmul(
                        out=ht[:], lhsT=ws[s][:], rhs=GT[:, s, :],
                        start=(s == 0), stop=(s == 6),
                    )
                tsum = psq.tile([C, 2], F32, tag="tsum")
                sq = psq.tile([C, SUP], F16, tag="sq")
                nc.scalar.activation(out=sq[:], in_=ht[:], func=AF.Square,
                                     accum_out=tsum[:, 1:2])
                emit_tail(t, ht, tsum)
                nc.vector.tensor_tensor(out=ssum[:], in0=ssum[:], in1=tsum[:],
                                        op=ALU.add)

            with tc.For_i(0, T, 1) as t:
                body(t)

        def tail_conv1(t, ht, tsum):
            # ht -> fp16 rows -> h1loc (raw h1, row-major)
            hsb = phsb.tile([C, SUP], F16, tag="hsb1")
            nc.scalar.activation(out=hsb[:], in_=ht[:], func=AF.Identity,
                                 accum_out=tsum[:, 0:1])
            pr = prow.tile([128, 4 * C], F16, tag="pr")
            for j in range(4):
                nc.tensor.transpose(
                    out=pr[:, j * C:(j + 1) * C],
                    in_=hsb[:, j * 128:(j + 1) * 128],
                    identity=id16[0:C, 0:C],
                )
            rowt = phsb.tile([128, 4, C], F16, tag="rowt")
            nc.vector.tensor_copy(
                out=rowt[:], in_=pr[:].rearrange("p (j c) -> p j c", c=C))
            nc.sync.dma_start(
                out=h1loc[bass.ds(t * SUP, SUP), :]
                    .rearrange("(j p) c -> p j c", p=128),
                in_=rowt[:])

        def tail_conv2(t, ht, tsum):
            hsb = phsb.tile([C, SUP], F16, tag="hsb2")
            nc.scalar.activation(out=hsb[:], in_=ht[:], func=AF.Identity,
                                 accum_out=tsum[:, 0:1])
            nc.sync.dma_start(out=scr2[:, bass.ts(t, SUP)], in_=hsb[:])

        def bn_finalize(sacc, stid, stod, gbt, ab):
            nc.sync.dma_start(out=stid[:], in_=sacc[:])
            nc.gpsimd.collective_compute(
                "AllReduce", ALU.add, replica_groups=groups,
                ins=[stid[:]], outs=[stod[:]],
            )
            stg = psc.tile([C, 2], F32, tag="stg")
            nc.sync.dma_start(out=stg[:], in_=stod[:])
            w = psc.tile([C, 8], F32, tag="work")
            mean, ex2, msq, var, veps, sd, rsd, ma = (w[:, i:i + 1] for i in range(8))
            nc.vector.tensor_scalar_mul(mean, stg[:, 0:1], 1.0 / n_true)
            nc.vector.tensor_scalar_mul(ex2, stg[:, 1:2], 1.0 / n_true)
            nc.vector.tensor_tensor(out=msq, in0=mean, in1=mean, op=ALU.mult)
            nc.vector.tensor_tensor(out=var, in0=ex2, in1=msq, op=ALU.subtract)
            nc.vector.tensor_scalar_add(veps, var, EPS)
            nc.scalar.sqrt(out=sd, in_=veps)
            nc.vector.reciprocal(out=rsd, in_=sd)
            nc.vector.tensor_tensor(out=ab[:, 0:1], in0=rsd, in1=gbt[:, 0:1], op=ALU.mult)
            nc.vector.tensor_tensor(out=ma, in0=mean, in1=ab[:, 0:1], op=ALU.mult)
            nc.vector.tensor_tensor(out=ab[:, 1:2], in0=gbt[:, 1:2], in1=ma, op=ALU.subtract)

        def conv_passB2():
            def body(t):
                hl = phsb.tile([C, SUP], F16, tag="hl")
                nc.sync.dma_start(out=hl[:], in_=scr2[:, bass.ts(t, SUP)])
                hr = psq.tile([C, SUP], F32, tag="hr")
                nc.scalar.activation(out=hr[:], in_=hl[:], func=AF.Identity,
                                     scale=ab2[:, 0:1], bias=ab2[:, 1:2])
                ptb = prow.tile([128, 4 * C], F32, tag="ptb")
                for j in range(4):
                    nc.tensor.transpose(
                        out=ptb[:, j * C:(j + 1) * C],
                        in_=hr[:, j * 128:(j + 1) * 128],
                        identity=id32[:],
                    )
                xt = phsb.tile([128, 4, C], F32, tag="xt")
                nc.sync.dma_start(
                    out=xt[:],
                    in_=xres[bass.ds(t * SUP, SUP), :]
                        .rearrange("(j p) c -> p j c", p=128))
                rowt = phsb.tile([128, 4, C], F32, tag="rowto")
                nc.vector.tensor_tensor(
                    out=rowt[:],
                    in0=ptb[:].rearrange("p (j c) -> p j c", c=C),
                    in1=xt[:], op=ALU.add)
                nc.vector.tensor_scalar_max(rowt[:], rowt[:], 0.0)
                nc.sync.dma_start(
                    out=outd[bass.ds(t * SUP, SUP), :]
                        .rearrange("(j p) c -> p j c", p=128),
                    in_=rowt[:])

            with tc.For_i(0, T, 1) as t:
                body(t)

        # ---- conv1: gather x, GEMM W1, emit raw h1 rows ----
        with nc.named_scope("passA1"):
            conv_passA(idx1, xt16, 0.0, w1s, None, sacc1, tail_conv1)
        with nc.named_scope("allgather"):
            nc.gpsimd.collective_compute(
                "AllGather", ALU.bypass, replica_groups=groups,
                ins=[h1loc[:]], outs=[h1ag[0:tbl2, :]],
            )
        with nc.named_scope("bn1"):
            bn_finalize(sacc1, st1i, st1o, gb1t, ab1)
            nc.sync.dma_start(out=ab1d[:], in_=ab1[:])
            for q in range(4):
                nc.sync.dma_start(out=ab1f[q * C:(q + 1) * C, :], in_=ab1d[:])
        # ---- conv2: gather raw h1, fused bn1+relu, GEMM W2 ----
        with nc.named_scope("passA2"):
            conv_passA(idx2, h1ag, NEG, w2s, ab1f, sacc2, tail_conv2)
        with nc.named_scope("bn2"):
            bn_finalize(sacc2, st2i, st2o, gb2t, ab2)
        # ---- bn2 + residual + relu ----
        with nc.named_scope("passB2"):
            conv_passB2()

        if debug_taps:
            taps = dict(h1loc=h1loc, scr2=scr2, st1o=st1o, st2o=st2o)
            for nm, tt in taps.items():
                d = nc.declare_dram_parameter(
                    f"dbg_{nm}", list(tt.shape),
                    F16 if nm in ("h1loc", "scr2") else F32, isOutput=True)
                nc.gpsimd.dma_start(out=d[:], in_=tt[:])
            dab = nc.declare_dram_parameter("dbg_ab1f", [128, 2], F32,
                                            isOutput=True)
            nc.gpsimd.dma_start(out=dab[:], in_=ab1f[:])

    nc.finalize()
    return nc


def host_prepare(x, nbr1, mask1, nbr2, mask2, W1, W2,
                 gamma1, beta1, gamma2, beta2,
                 rows_pc, R, T, n_cores=N_CORES):
    """Translate indices / pad / pack per-core input maps."""
    n = x.shape[0]
    assert gamma1.min() > 0, "fused bn1 relu trick needs gamma1 > 0"

    iv1 = np.where(mask1 != 0, nbr1, n).astype(np.int32)
    c_of = (nbr2.astype(np.int64)) // rows_pc
    pos2 = c_of * R + (nbr2.astype(np.int64) - c_of * rows_pc)
    iv2 = np.where(mask2 != 0, pos2, N_CORES * R).astype(np.int32)

    xt16 = np.zeros((n + 1, C), np.float16)
    xt16[:n] = x.astype(np.float16)

    def wcat(W):
        w = np.zeros((KP * C, C), np.float16)
        w[:K * C] = W.reshape(K * C, C)
        return w

    w1cat, w2cat = wcat(W1), wcat(W2)
    gb1 = np.stack([gamma1, beta1]).astype(np.float32)
    gb2 = np.stack([gamma2, beta2]).astype(np.float32)

    def pack_idx(iv, zfill, c):
        a = iv[c * rows_pc:(c + 1) * rows_pc]        # [rows_pc, 27]
        ap = np.full((R, KP), zfill, np.int32)
        ap[:rows_pc, :K] = a
        # row (t, j, p) = t*512 + j*128 + p ; columns (j, k)
        ap = ap.reshape(T, 4, 128, KP).transpose(0, 2, 1, 3).reshape(T, 128, 4 * KP)
        return np.ascontiguousarray(ap)

    in_maps = []
    for c in range(n_cores):
        xr = np.zeros((R, C), np.float32)
        xr[:rows_pc] = x[c * rows_pc:(c + 1) * rows_pc]
        in_maps.append(dict(
            xt16=xt16,
            xres=xr,
            idx1=pack_idx(iv1, n, c),
            idx2=pack_idx(iv2, N_CORES * R, c),
            w1cat=w1cat, w2cat=w2cat, gb1=gb1, gb2=gb2,
        ))
    return in_maps


_CACHE = {}


def _get_program():
    key = (R_FULL, T_FULL)
    if key not in _CACHE:
        _CACHE[key] = build_program(R_FULL, T_FULL, N_FULL)
    return _CACHE[key]


def kernel(x, nbr1, mask1, nbr2, mask2, W1, W2, gamma1, beta1, gamma2, beta2,
           _trace=False, _trace_kwargs=None):
    x = np.asarray(x, np.float32)
    in_maps = host_prepare(
        x, np.asarray(nbr1), np.asarray(mask1), np.asarray(nbr2),
        np.asarray(mask2), np.asarray(W1, np.float32), np.asarray(W2, np.float32),
        np.asarray(gamma1, np.float32), np.asarray(beta1, np.float32),
        np.asarray(gamma2, np.float32), np.asarray(beta2, np.float32),
        ROWS_PC, R_FULL, T_FULL)
    nc = _get_program()
    res = run_bass_kernel_spmd(nc, in_maps, list(range(N_CORES)),
                               trace=_trace, **(_trace_kwargs or {}))
    out = np.concatenate(
        [res.results[c]["out"][:ROWS_PC] for c in range(N_CORES)], axis=0)
    if _trace:
        return out, res
    return out


# revision 10
# speedup vs baseline: 1.0042x; 1.0042x over previous
"""Trainium2 Bass kernel: Minkowski-style sparse-conv BasicBlock.

  out = relu(bn2(conv2(relu(bn1(conv1(x))))) + x)

conv(x)[i] = sum_k mask[i,k] * x[nbr[i,k]] @ W[k]   (K=27 offsets, C=32 ch)
BN is training-mode (batch statistics over all N=1e6 rows).

Sharding: data-parallel over points across 8 NeuronCores; gather tables
replicated (x) / AllGather'd (raw h1).

Key design points vs a naive implementation:
 - ONE batched indirect DMA per 512-row supertile (14336 indices) instead
   of one per (k, j-block): amortizes the ~1us SWDGE per-call fixed cost.
 - fp16 tables + fp16 GEMM path (fp32 PSUM accumulate): halves gather
   bytes and runs the PE at full rate.
 - masked edges are skipped entirely via bounds_check (index sentinel
   0x7FFFFF00 > bounds): ~50% fewer gather descriptors. G tiles are
   pre-filled with 0 (conv1) / -60000 (conv2) so skipped slots contribute
   nothing after the (monotone) BN affine + relu.
 - conv1 has NO second pass: raw (pre-BN) h1 is written row-major and
   AllGather'd during pass A1; BN1+ReLU is folded into conv2 as a
   per-partition affine+relu on the transposed slabs (channel lives on
   partitions there), fused with the PSUM->SBUF copy on the ACT engine.
   Requires sign(gamma1) uniform (gamma1=1 here): sentinel slots then
   relu to exactly 0.

Per-core pass A supertile pipeline (512 rows):
  idx load [128,112] -> fill G -> 1 indirect gather (skip-masked) ->
  28 PE transposes -> (conv2: fused affine+relu) -> 7 fp16 matmuls ->
  ht[32,512] PSUM -> ACT copy (+sum/sumsq accum for BN) ->
  conv1: transpose-back to rows -> h1loc; conv2: -> scr2 channel-major.
BN stats AllReduce'd ([32,2]); pass B2 applies bn2 + residual + relu.
"""

import numpy as np
from contextlib import ExitStack

import concourse.bass as bass
import concourse.tile as tile
import concourse.mybir as mybir
from concourse import bacc
from concourse.bass import IndirectOffsetOnAxis
from concourse.masks import make_identity
from concourse.bass_utils import run_bass_kernel_spmd

F32 = mybir.dt.float32
F16 = mybir.dt.float16
I32 = mybir.dt.int32
AF = mybir.ActivationFunctionType
ALU = mybir.AluOpType

N_FULL = 1_000_000
C = 32
K = 27
KP = 28            # k padded to 28 => 28*32 = 896 = 7 slabs of 128
EPS = 1e-5
N_CORES = 8
SUP = 512          # rows per supertile
ROWS_PC = N_FULL // N_CORES            # 125000
T_FULL = -(-ROWS_PC // SUP)            # 245
R_FULL = T_FULL * SUP                  # 125440
OOB = 0x7FFFFF00   # skip sentinel (> any bounds_check value)
NEG = -60000.0     # conv2 G fill; relu(a*NEG+b) == 0 for a > ~1e-3


def build_program(R, T, n_true, n_cores=N_CORES, debug_taps=False, reps=1):
    """Build the SPMD Bass program (identical on every core)."""
    assert R == T * SUP
    tbl2 = n_cores * R
    nc = bacc.Bacc(None, num_devices=n_cores)

    xt16 = nc.declare_dram_parameter("xt16", [N_FULL + 1, C], F16, isOutput=False)
    xres = nc.declare_dram_parameter("xres", [R, C], F32, isOutput=False)
    idx1 = nc.declare_dram_parameter("idx1", [T, 128, 4 * KP], I32, isOutput=False)
    idx2 = nc.declare_dram_parameter("idx2", [T, 128, 4 * KP], I32, isOutput=False)
    w1c = nc.declare_dram_parameter("w1cat", [KP * C, C], F16, isOutput=False)
    w2c = nc.declare_dram_parameter("w2cat", [KP * C, C], F16, isOutput=False)
    gb1 = nc.declare_dram_parameter("gb1", [2, C], F32, isOutput=False)
    gb2 = nc.declare_dram_parameter("gb2", [2, C], F32, isOutput=False)
    outd = nc.declare_dram_parameter("out", [R, C], F32, isOutput=True)

    scr2 = nc.dram_tensor("scr2", [C, R], F16)
    h1loc = nc.dram_tensor("h1loc", [R, C], F16)
    h1ag = nc.dram_tensor("h1ag", [tbl2 + 1, C], F16, addr_space="Shared")
    st1i = nc.dram_tensor("st1i", [C, 2], F32)
    st1o = nc.dram_tensor("st1o", [C, 2], F32, addr_space="Shared")
    st2i = nc.dram_tensor("st2i", [C, 2], F32)
    st2o = nc.dram_tensor("st2o", [C, 2], F32, addr_space="Shared")
    ab1d = nc.dram_tensor("ab1d", [C, 2], F32)

    groups = [list(range(n_cores))]
    NI = 4 * KP * 128          # indices per supertile gather

    with ExitStack() as ctx:
        tc = ctx.enter_context(tile.TileContext(nc))
        cpool = ctx.enter_context(tc.tile_pool(name="const", bufs=1))
        id16 = cpool.tile([128, 128], F16, tag="id16")
        make_identity(nc, id16[:])
        id32 = cpool.tile([C, C], F32, tag="id32")
        make_identity(nc, id32[:])

        def load_w(wc, nm):
            ws = []
            for s in range(7):
                wt = cpool.tile([128, C], F16, tag=f"{nm}_{s}")
                nc.sync.dma_start(out=wt[:], in_=wc[s * 128:(s + 1) * 128, :])
                ws.append(wt)
            return ws

        w1s = load_w(w1c, "w1")
        w2s = load_w(w2c, "w2")

        gb1t = cpool.tile([C, 2], F32, tag="gb1")
        nc.sync.dma_start(out=gb1t[:], in_=gb1[:].rearrange("a c -> c a"))
        gb2t = cpool.tile([C, 2], F32, tag="gb2")
        nc.sync.dma_start(out=gb2t[:], in_=gb2[:].rearrange("a c -> c a"))

        negrow = cpool.tile([1, C], F16, tag="negrow")
        nc.vector.memset(negrow[:], NEG)
        nc.gpsimd.dma_start(out=h1ag[tbl2:tbl2 + 1, :], in_=negrow[:])

        stat = ctx.enter_context(tc.tile_pool(name="stat", bufs=1))
        sacc1 = stat.tile([C, 2], F32, tag="sacc1")
        sacc2 = stat.tile([C, 2], F32, tag="sacc2")
        ab1 = stat.tile([C, 2], F32, tag="ab1")
        ab2 = stat.tile([C, 2], F32, tag="ab2")
        ab1f = stat.tile([128, 2], F32, tag="ab1f")   # ab1 tiled 4x on partitions

        pidx = ctx.enter_context(tc.tile_pool(name="pidx", bufs=3))
        pg = ctx.enter_context(tc.tile_pool(name="pg", bufs=3))
        pgt = ctx.enter_context(tc.tile_pool(name="pgt", bufs=2))
        ppt = ctx.enter_context(tc.tile_pool(name="ppt", bufs=3, space="PSUM"))
        pht = ctx.enter_context(tc.tile_pool(name="pht", bufs=2, space="PSUM"))
        prow = ctx.enter_context(tc.tile_pool(name="prow", bufs=1, space="PSUM"))
        phsb = ctx.enter_context(tc.tile_pool(name="phsb", bufs=3))
        psq = ctx.enter_context(tc.tile_pool(name="psq", bufs=2))
        psc = ctx.enter_context(tc.tile_pool(name="psc", bufs=1))

        def conv_passA(idxd, table, fill, ws, affine, ssum, emit_tail):
            """One supertile pass: gather -> transpose -> [affine] -> GEMM.

            emit_tail(t, ht) consumes the [C, SUP] fp32 PSUM result."""
            nc.vector.memset(ssum[:], 0.0)

            def body(t):
                idxt = pidx.tile([128, 4 * KP], I32, tag="idx")
                nc.sync.dma_start(out=idxt[:],
                                  in_=idxd[bass.ds(t, 1), :, :].squeeze(0))
                G = pg.tile([128, 4 * KP, C], F16, tag="g")
                for j in range(4):
                    nc.vector.memset(G[:, j * KP + K, :], fill)  # pad col
                    for k in range(K):
                        c = j * KP + k
                        nc.gpsimd.indirect_dma_start(
                            out=G[:, c, :], out_offset=None,
                            in_=table[:],
                            in_offset=IndirectOffsetOnAxis(
                                ap=idxt[:, c:c + 1], axis=0),
                        )
                Gf = G[:].rearrange("p a c -> p (a c)")
                GT = pgt.tile([128, 7, SUP], F16, tag="gt")
                for s in range(7):
                    pt = ppt.tile([128, SUP], F16, tag="pt")
                    for j in range(4):
                        nc.tensor.transpose(
                            out=pt[:, j * 128:(j + 1) * 128],
                            in_=Gf[:, j * KP * C + s * 128:
                                   j * KP * C + s * 128 + 128],
                            identity=id16[:],
                        )
                    if affine is not None:
                        # fused PSUM->SBUF copy + bn1 affine + relu
                        nc.scalar.activation(
                            out=GT[:, s, :], in_=pt[:], func=AF.Relu,
                            scale=affine[:, 0:1], bias=affine[:, 1:2])
                    elif s % 2 == 0:
                        nc.scalar.copy(out=GT[:, s, :], in_=pt[:])
                    else:
                        nc.vector.tensor_copy(out=GT[:, s, :], in_=pt[:])
                ht = pht.tile([C, SUP], F32, tag="ht")
                for s in range(7):
                    nc.tensor.matmul(
                        out=ht[:], lhsT=ws[s][:], rhs=GT[:, s, :],
                        start=(s == 0), stop=(s == 6),
                    )
                tsum = psq.tile([C, 2], F32, tag="tsum")
                sq = psq.tile([C, SUP], F16, tag="sq")
                nc.scalar.activation(out=sq[:], in_=ht[:], func=AF.Square,
                                     accum_out=tsum[:, 1:2])
                emit_tail(t, ht, tsum)
                nc.vector.tensor_tensor(out=ssum[:], in0=ssum[:], in1=tsum[:],
                                        op=ALU.add)

            with tc.For_i(0, T, 1) as t:
                body(t)

        def tail_conv1(t, ht, tsum):
            # ht -> fp16 rows -> h1loc (raw h1, row-major)
            hsb = phsb.tile([C, SUP], F16, tag="hsb1")
            nc.scalar.activation(out=hsb[:], in_=ht[:], func=AF.Identity,
                                 accum_out=tsum[:, 0:1])
            pr = prow.tile([128, 4 * C], F16, tag="pr")
            for j in range(4):
                nc.tensor.transpose(
                    out=pr[:, j * C:(j + 1) * C],
                    in_=hsb[:, j * 128:(j + 1) * 128],
                    identity=id16[0:C, 0:C],
                )
            rowt = phsb.tile([128, 4, C], F16, tag="rowt")
            nc.vector.tensor_copy(
                out=rowt[:], in_=pr[:].rearrange("p (j c) -> p j c", c=C))
            nc.sync.dma_start(
                out=h1loc[bass.ds(t * SUP, SUP), :]
                    .rearrange("(j p) c -> p j c", p=128),
                in_=rowt[:])

        def tail_conv2(t, ht, tsum):
            hsb = phsb.tile([C, SUP], F16, tag="hsb2")
            nc.scalar.activation(out=hsb[:], in_=ht[:], func=AF.Identity,
                                 accum_out=tsum[:, 0:1])
            nc.sync.dma_start(out=scr2[:, bass.ts(t, SUP)], in_=hsb[:])

        def bn_finalize(sacc, stid, stod, gbt, ab):
            nc.sync.dma_start(out=stid[:], in_=sacc[:])
            nc.gpsimd.collective_compute(
                "AllReduce", ALU.add, replica_groups=groups,
                ins=[stid[:]], outs=[stod[:]],
            )
            stg = psc.tile([C, 2], F32, tag="stg")
            nc.sync.dma_start(out=stg[:], in_=stod[:])
            w = psc.tile([C, 8], F32, tag="work")
            mean, ex2, msq, var, veps, sd, rsd, ma = (w[:, i:i + 1] for i in range(8))
            nc.vector.tensor_scalar_mul(mean, stg[:, 0:1], 1.0 / n_true)
            nc.vector.tensor_scalar_mul(ex2, stg[:, 1:2], 1.0 / n_true)
            nc.vector.tensor_tensor(out=msq, in0=mean, in1=mean, op=ALU.mult)
            nc.vector.tensor_tensor(out=var, in0=ex2, in1=msq, op=ALU.subtract)
            nc.vector.tensor_scalar_add(veps, var, EPS)
            nc.scalar.sqrt(out=sd, in_=veps)
            nc.vector.reciprocal(out=rsd, in_=sd)
            nc.vector.tensor_tensor(out=ab[:, 0:1], in0=rsd, in1=gbt[:, 0:1], op=ALU.mult)
            nc.vector.tensor_tensor(out=ma, in0=mean, in1=ab[:, 0:1], op=ALU.mult)
            nc.vector.tensor_tensor(out=ab[:, 1:2], in0=gbt[:, 1:2], in1=ma, op=ALU.subtract)

        def conv_passB2():
            def body(t):
                hl = phsb.tile([C, SUP], F16, tag="hl")
                nc.sync.dma_start(out=hl[:], in_=scr2[:, bass.ts(t, SUP)])
                hr = psq.tile([C, SUP], F32, tag="hr")
                nc.scalar.activation(out=hr[:], in_=hl[:], func=AF.Identity,
                                     scale=ab2[:, 0:1], bias=ab2[:, 1:2])
                ptb = prow.tile([128, 4 * C], F32, tag="ptb")
                for j in range(4):
                    nc.tensor.transpose(
                        out=ptb[:, j * C:(j + 1) * C],
                        in_=hr[:, j * 128:(j + 1) * 128],
                        identity=id32[:],
                    )
                xt = phsb.tile([128, 4, C], F32, tag="xt")
                nc.sync.dma_start(
                    out=xt[:],
                    in_=xres[bass.ds(t * SUP, SUP), :]
                        .rearrange("(j p) c -> p j c", p=128))
                rowt = phsb.tile([128, 4, C], F32, tag="rowto")
                nc.vector.tensor_tensor(
                    out=rowt[:],
                    in0=ptb[:].rearrange("p (j c) -> p j c", c=C),
                    in1=xt[:], op=ALU.add)
                nc.vector.tensor_scalar_max(rowt[:], rowt[:], 0.0)
                nc.sync.dma_start(
                    out=outd[bass.ds(t * SUP, SUP), :]
                        .rearrange("(j p) c -> p j c", p=128),
                    in_=rowt[:])

            with tc.For_i(0, T, 1) as t:
                body(t)

        def run_pipeline():
            # ---- conv1: gather x, GEMM W1, emit raw h1 rows ----
            with nc.named_scope("passA1"):
                conv_passA(idx1, xt16, 0.0, w1s, None, sacc1, tail_conv1)
            with nc.named_scope("allgather"):
                nc.gpsimd.collective_compute(
                    "AllGather", ALU.bypass, replica_groups=groups,
                    ins=[h1loc[:]], outs=[h1ag[0:tbl2, :]],
                )
            with nc.named_scope("bn1"):
                bn_finalize(sacc1, st1i, st1o, gb1t, ab1)
                nc.sync.dma_start(out=ab1d[:], in_=ab1[:])
                for q in range(4):
                    nc.sync.dma_start(out=ab1f[q * C:(q + 1) * C, :], in_=ab1d[:])
            # ---- conv2: gather raw h1, fused bn1+relu, GEMM W2 ----
            with nc.named_scope("passA2"):
                conv_passA(idx2, h1ag, NEG, w2s, ab1f, sacc2, tail_conv2)
            with nc.named_scope("bn2"):
                bn_finalize(sacc2, st2i, st2o, gb2t, ab2)
            # ---- bn2 + residual + relu ----
            with nc.named_scope("passB2"):
                conv_passB2()

        for _rep in range(reps):
            run_pipeline()

        if debug_taps:
            taps = dict(h1loc=h1loc, scr2=scr2, st1o=st1o, st2o=st2o)
            for nm, tt in taps.items():
                d = nc.declare_dram_parameter(
                    f"dbg_{nm}", list(tt.shape),
                    F16 if nm in ("h1loc", "scr2") else F32, isOutput=True)
                nc.gpsimd.dma_start(out=d[:], in_=tt[:])
            dab = nc.declare_dram_parameter("dbg_ab1f", [128, 2], F32,
                                            isOutput=True)
            nc.gpsimd.dma_start(out=dab[:], in_=ab1f[:])

    nc.finalize()
    return nc


def host_prepare(x, nbr1, mask1, nbr2, mask2, W1, W2,
                 gamma1, beta1, gamma2, beta2,
                 rows_pc, R, T, n_cores=N_CORES):
    """Translate indices / pad / pack per-core input maps."""
    n = x.shape[0]
    assert gamma1.min() > 0, "fused bn1 relu trick needs gamma1 > 0"

    iv1 = np.where(mask1 != 0, nbr1, n).astype(np.int32)
    c_of = (nbr2.astype(np.int64)) // rows_pc
    pos2 = c_of * R + (nbr2.astype(np.int64) - c_of * rows_pc)
    iv2 = np.where(mask2 != 0, pos2, N_CORES * R).astype(np.int32)

    xt16 = np.zeros((n + 1, C), np.float16)
    xt16[:n] = x.astype(np.float16)

    def wcat(W):
        w = np.zeros((KP * C, C), np.float16)
        w[:K * C] = W.reshape(K * C, C)
        return w

    w1cat, w2cat = wcat(W1), wcat(W2)
    gb1 = np.stack([gamma1, beta1]).astype(np.float32)
    gb2 = np.stack([gamma2, beta2]).astype(np.float32)

    def pack_idx(iv, zfill, c):
        a = iv[c * rows_pc:(c + 1) * rows_pc]        # [rows_pc, 27]
        ap = np.full((R, KP), zfill, np.int32)
        ap[:rows_pc, :K] = a
        # row (t, j, p) = t*512 + j*128 + p ; columns (j, k)
        ap = ap.reshape(T, 4, 128, KP).transpose(0, 2, 1, 3).reshape(T, 128, 4 * KP)
        return np.ascontiguousarray(ap)

    in_maps = []
    for c in range(n_cores):
        xr = np.zeros((R, C), np.float32)
        xr[:rows_pc] = x[c * rows_pc:(c + 1) * rows_pc]
        in_maps.append(dict(
            xt16=xt16,
            xres=xr,
            idx1=pack_idx(iv1, n, c),
            idx2=pack_idx(iv2, N_CORES * R, c),
            w1cat=w1cat, w2cat=w2cat, gb1=gb1, gb2=gb2,
        ))
    return in_maps


_CACHE = {}


def _get_program():
    key = (R_FULL, T_FULL)
    if key not in _CACHE:
        _CACHE[key] = build_program(R_FULL, T_FULL, N_FULL)
    return _CACHE[key]


def kernel(x, nbr1, mask1, nbr2, mask2, W1, W2, gamma1, beta1, gamma2, beta2,
           _trace=False, _trace_kwargs=None):
    x = np.asarray(x, np.float32)
    in_maps = host_prepare(
        x, np.asarray(nbr1), np.asarray(mask1), np.asarray(nbr2),
        np.asarray(mask2), np.asarray(W1, np.float32), np.asarray(W2, np.float32),
        np.asarray(gamma1, np.float32), np.asarray(beta1, np.float32),
        np.asarray(gamma2, np.float32), np.asarray(beta2, np.float32),
        ROWS_PC, R_FULL, T_FULL)
    nc = _get_program()
    res = run_bass_kernel_spmd(nc, in_maps, list(range(N_CORES)),
                               trace=_trace, **(_trace_kwargs or {}))
    out = np.concatenate(
        [res.results[c]["out"][:ROWS_PC] for c in range(N_CORES)], axis=0)
    if _trace:
        return out, res
    return out


# revision 12
# speedup vs baseline: 1.0555x; 1.0511x over previous
"""Trainium2 Bass kernel: Minkowski-style sparse-conv BasicBlock.

  out = relu(bn2(conv2(relu(bn1(conv1(x))))) + x)

conv(x)[i] = sum_k mask[i,k] * x[nbr[i,k]] @ W[k]   (K=27 offsets, C=32 ch)
BN is training-mode (batch statistics over all N=1e6 rows).

Sharding: data-parallel over points across 8 NeuronCores; gather tables
replicated (x) / AllGather'd (raw h1). Masked / padded edges are
redirected host-side to a zero row (conv1) or a -60000 sentinel row
(conv2) appended to the gather table.

Key design points:
 - fp16 tables + fp16 transpose/GEMM path with fp32 PSUM accumulation
   (rel err ~5e-4 vs the 2e-2 gate): halves gather bytes, full PE rate.
 - gathers use the one-index-per-partition indirect-DMA shape
   ([128,1] offsets -> [128,32] rows), the only shape the hardware DGE
   unrolls correctly; one call per (j-block, k) column, 108 per
   512-row supertile. The ~1.5us/call SWDGE cost on the Pool engine is
   the kernel's dominant serial resource; all other engines (PE
   transposes+GEMMs, ACT copies/stats, DVE memsets, SDMA drains) hide
   under it via tile pipelining.
 - conv1 has NO second pass: raw (pre-BN) h1 is transposed back to
   row-major fp16 inside pass A and AllGather'd; BN1+ReLU is folded
   into conv2 as a per-partition affine+relu on the transposed slabs
   (channel = partition mod 32 there), fused with the PSUM->SBUF copy
   on the ACT engine. The -60000 sentinel then relus to exactly 0
   (requires gamma1 > 0, asserted host-side; true for these inputs).
 - BN stats ([32,2] sum/sumsq) accumulate via ACT accum_out and one
   tiny AllReduce; pass B2 applies bn2 + residual + relu.

build_program(reps=M) repeats the whole pipeline M times for
slope-based timing (cancels the multi-ms PJRT dispatch overhead).
"""

import numpy as np
from contextlib import ExitStack

import concourse.bass as bass
import concourse.tile as tile
import concourse.mybir as mybir
from concourse import bacc
from concourse.bass import IndirectOffsetOnAxis
from concourse.masks import make_identity
from concourse.bass_utils import run_bass_kernel_spmd

F32 = mybir.dt.float32
F16 = mybir.dt.float16
I32 = mybir.dt.int32
AF = mybir.ActivationFunctionType
ALU = mybir.AluOpType

N_FULL = 1_000_000
C = 32
K = 27
KP = 28            # k padded to 28 => 28*32 = 896 = 7 slabs of 128
EPS = 1e-5
N_CORES = 8
SUP = 512          # rows per supertile
ROWS_PC = N_FULL // N_CORES            # 125000
T_FULL = -(-ROWS_PC // SUP)            # 245
R_FULL = T_FULL * SUP                  # 125440
OOB = 0x7FFFFF00   # skip sentinel (> any bounds_check value)
NEG = -60000.0     # conv2 G fill; relu(a*NEG+b) == 0 for a > ~1e-3


def build_program(R, T, n_true, n_cores=N_CORES, debug_taps=False, reps=1):
    """Build the SPMD Bass program (identical on every core)."""
    assert R == T * SUP
    tbl2 = n_cores * R
    nc = bacc.Bacc(None, num_devices=n_cores)

    xt16 = nc.declare_dram_parameter("xt16", [N_FULL + 1, C], F16, isOutput=False)
    xres = nc.declare_dram_parameter("xres", [R, C], F32, isOutput=False)
    idx1 = nc.declare_dram_parameter("idx1", [T, 128, 4 * KP], I32, isOutput=False)
    idx2 = nc.declare_dram_parameter("idx2", [T, 128, 4 * KP], I32, isOutput=False)
    w1c = nc.declare_dram_parameter("w1cat", [KP * C, C], F16, isOutput=False)
    w2c = nc.declare_dram_parameter("w2cat", [KP * C, C], F16, isOutput=False)
    gb1 = nc.declare_dram_parameter("gb1", [2, C], F32, isOutput=False)
    gb2 = nc.declare_dram_parameter("gb2", [2, C], F32, isOutput=False)
    outd = nc.declare_dram_parameter("out", [R, C], F32, isOutput=True)

    scr2 = nc.dram_tensor("scr2", [C, R], F16)
    h1loc = nc.dram_tensor("h1loc", [R, C], F16)
    h1ag = nc.dram_tensor("h1ag", [tbl2 + 1, C], F16, addr_space="Shared")
    st1i = nc.dram_tensor("st1i", [C, 2], F32)
    st1o = nc.dram_tensor("st1o", [C, 2], F32, addr_space="Shared")
    st2i = nc.dram_tensor("st2i", [C, 2], F32)
    st2o = nc.dram_tensor("st2o", [C, 2], F32, addr_space="Shared")
    ab1d = nc.dram_tensor("ab1d", [C, 2], F32)

    groups = [list(range(n_cores))]
    NI = 4 * KP * 128          # indices per supertile gather

    with ExitStack() as ctx:
        tc = ctx.enter_context(tile.TileContext(nc))
        cpool = ctx.enter_context(tc.tile_pool(name="const", bufs=1))
        id16 = cpool.tile([128, 128], F16, tag="id16")
        make_identity(nc, id16[:])
        id32 = cpool.tile([C, C], F32, tag="id32")
        make_identity(nc, id32[:])

        def load_w(wc, nm):
            ws = []
            for s in range(7):
                wt = cpool.tile([128, C], F16, tag=f"{nm}_{s}")
                nc.sync.dma_start(out=wt[:], in_=wc[s * 128:(s + 1) * 128, :])
                ws.append(wt)
            return ws

        w1s = load_w(w1c, "w1")
        w2s = load_w(w2c, "w2")

        gb1t = cpool.tile([C, 2], F32, tag="gb1")
        nc.sync.dma_start(out=gb1t[:], in_=gb1[:].rearrange("a c -> c a"))
        gb2t = cpool.tile([C, 2], F32, tag="gb2")
        nc.sync.dma_start(out=gb2t[:], in_=gb2[:].rearrange("a c -> c a"))

        negrow = cpool.tile([1, C], F16, tag="negrow")
        nc.vector.memset(negrow[:], NEG)
        nc.gpsimd.dma_start(out=h1ag[tbl2:tbl2 + 1, :], in_=negrow[:])

        stat = ctx.enter_context(tc.tile_pool(name="stat", bufs=1))
        sacc1 = stat.tile([C, 2], F32, tag="sacc1")
        sacc2 = stat.tile([C, 2], F32, tag="sacc2")
        ab1 = stat.tile([C, 2], F32, tag="ab1")
        ab2 = stat.tile([C, 2], F32, tag="ab2")
        ab1f = stat.tile([128, 2], F32, tag="ab1f")   # ab1 tiled 4x on partitions

        pidx = ctx.enter_context(tc.tile_pool(name="pidx", bufs=4))
        pg = ctx.enter_context(tc.tile_pool(name="pg", bufs=4))
        pgt = ctx.enter_context(tc.tile_pool(name="pgt", bufs=2))
        ppt = ctx.enter_context(tc.tile_pool(name="ppt", bufs=3, space="PSUM"))
        pht = ctx.enter_context(tc.tile_pool(name="pht", bufs=2, space="PSUM"))
        prow = ctx.enter_context(tc.tile_pool(name="prow", bufs=1, space="PSUM"))
        phsb = ctx.enter_context(tc.tile_pool(name="phsb", bufs=3))
        psq = ctx.enter_context(tc.tile_pool(name="psq", bufs=2))
        psc = ctx.enter_context(tc.tile_pool(name="psc", bufs=1))

        def conv_passA(idxd, table, fill, ws, affine, ssum, emit_tail):
            """One supertile pass: gather -> transpose -> [affine] -> GEMM.

            emit_tail(t, ht) consumes the [C, SUP] fp32 PSUM result."""
            nc.vector.memset(ssum[:], 0.0)

            def body(t):
                idxt = pidx.tile([128, 4 * KP], I32, tag="idx")
                nc.sync.dma_start(out=idxt[:],
                                  in_=idxd[bass.ds(t, 1), :, :].squeeze(0))
                G = pg.tile([128, 4 * KP, C], F16, tag="g")
                for j in range(4):
                    nc.vector.memset(G[:, j * KP + K, :], fill)  # pad col
                    for k in range(K):
                        c = j * KP + k
                        nc.gpsimd.indirect_dma_start(
                            out=G[:, c, :], out_offset=None,
                            in_=table[:],
                            in_offset=IndirectOffsetOnAxis(
                                ap=idxt[:, c:c + 1], axis=0),
                        )
                Gf = G[:].rearrange("p a c -> p (a c)")
                GT = pgt.tile([128, 7, SUP], F16, tag="gt")
                for s in range(7):
                    pt = ppt.tile([128, SUP], F16, tag="pt")
                    for j in range(4):
                        nc.tensor.transpose(
                            out=pt[:, j * 128:(j + 1) * 128],
                            in_=Gf[:, j * KP * C + s * 128:
                                   j * KP * C + s * 128 + 128],
                            identity=id16[:],
                        )
                    if affine is not None:
                        # fused PSUM->SBUF copy + bn1 affine + relu
                        nc.scalar.activation(
                            out=GT[:, s, :], in_=pt[:], func=AF.Relu,
                            scale=affine[:, 0:1], bias=affine[:, 1:2])
                    elif s % 2 == 0:
                        nc.scalar.copy(out=GT[:, s, :], in_=pt[:])
                    else:
                        nc.vector.tensor_copy(out=GT[:, s, :], in_=pt[:])
                ht = pht.tile([C, SUP], F32, tag="ht")
                for s in range(7):
                    nc.tensor.matmul(
                        out=ht[:], lhsT=ws[s][:], rhs=GT[:, s, :],
                        start=(s == 0), stop=(s == 6),
                    )
                tsum = psq.tile([C, 2], F32, tag="tsum")
                sq = psq.tile([C, SUP], F16, tag="sq")
                nc.scalar.activation(out=sq[:], in_=ht[:], func=AF.Square,
                                     accum_out=tsum[:, 1:2])
                emit_tail(t, ht, tsum)
                nc.vector.tensor_tensor(out=ssum[:], in0=ssum[:], in1=tsum[:],
                                        op=ALU.add)

            with tc.For_i(0, T, 1) as t:
                body(t)

        def tail_conv1(t, ht, tsum):
            # ht -> fp16 rows -> h1loc (raw h1, row-major)
            hsb = phsb.tile([C, SUP], F16, tag="hsb1")
            nc.scalar.activation(out=hsb[:], in_=ht[:], func=AF.Identity,
                                 accum_out=tsum[:, 0:1])
            pr = prow.tile([128, 4 * C], F16, tag="pr")
            for j in range(4):
                nc.tensor.transpose(
                    out=pr[:, j * C:(j + 1) * C],
                    in_=hsb[:, j * 128:(j + 1) * 128],
                    identity=id16[0:C, 0:C],
                )
            rowt = phsb.tile([128, 4, C], F16, tag="rowt")
            nc.vector.tensor_copy(
                out=rowt[:], in_=pr[:].rearrange("p (j c) -> p j c", c=C))
            nc.sync.dma_start(
                out=h1loc[bass.ds(t * SUP, SUP), :]
                    .rearrange("(j p) c -> p j c", p=128),
                in_=rowt[:])

        def tail_conv2(t, ht, tsum):
            hsb = phsb.tile([C, SUP], F16, tag="hsb2")
            nc.scalar.activation(out=hsb[:], in_=ht[:], func=AF.Identity,
                                 accum_out=tsum[:, 0:1])
            nc.sync.dma_start(out=scr2[:, bass.ts(t, SUP)], in_=hsb[:])

        def bn_finalize(sacc, stid, stod, gbt, ab):
            nc.sync.dma_start(out=stid[:], in_=sacc[:])
            nc.gpsimd.collective_compute(
                "AllReduce", ALU.add, replica_groups=groups,
                ins=[stid[:]], outs=[stod[:]],
            )
            stg = psc.tile([C, 2], F32, tag="stg")
            nc.sync.dma_start(out=stg[:], in_=stod[:])
            w = psc.tile([C, 8], F32, tag="work")
            mean, ex2, msq, var, veps, sd, rsd, ma = (w[:, i:i + 1] for i in range(8))
            nc.vector.tensor_scalar_mul(mean, stg[:, 0:1], 1.0 / n_true)
            nc.vector.tensor_scalar_mul(ex2, stg[:, 1:2], 1.0 / n_true)
            nc.vector.tensor_tensor(out=msq, in0=mean, in1=mean, op=ALU.mult)
            nc.vector.tensor_tensor(out=var, in0=ex2, in1=msq, op=ALU.subtract)
            nc.vector.tensor_scalar_add(veps, var, EPS)
            nc.scalar.sqrt(out=sd, in_=veps)
            nc.vector.reciprocal(out=rsd, in_=sd)
            nc.vector.tensor_tensor(out=ab[:, 0:1], in0=rsd, in1=gbt[:, 0:1], op=ALU.mult)
            nc.vector.tensor_tensor(out=ma, in0=mean, in1=ab[:, 0:1], op=ALU.mult)
            nc.vector.tensor_tensor(out=ab[:, 1:2], in0=gbt[:, 1:2], in1=ma, op=ALU.subtract)

        def conv_passB2():
            def body(t):
                hl = phsb.tile([C, SUP], F16, tag="hl")
                nc.sync.dma_start(out=hl[:], in_=scr2[:, bass.ts(t, SUP)])
                hr = psq.tile([C, SUP], F32, tag="hr")
                nc.scalar.activation(out=hr[:], in_=hl[:], func=AF.Identity,
                                     scale=ab2[:, 0:1], bias=ab2[:, 1:2])
                ptb = prow.tile([128, 4 * C], F32, tag="ptb")
                for j in range(4):
                    nc.tensor.transpose(
                        out=ptb[:, j * C:(j + 1) * C],
                        in_=hr[:, j * 128:(j + 1) * 128],
                        identity=id32[:],
                    )
                xt = phsb.tile([128, 4, C], F32, tag="xt")
                nc.sync.dma_start(
                    out=xt[:],
                    in_=xres[bass.ds(t * SUP, SUP), :]
                        .rearrange("(j p) c -> p j c", p=128))
                rowt = phsb.tile([128, 4, C], F32, tag="rowto")
                nc.vector.tensor_tensor(
                    out=rowt[:],
                    in0=ptb[:].rearrange("p (j c) -> p j c", c=C),
                    in1=xt[:], op=ALU.add)
                nc.vector.tensor_scalar_max(rowt[:], rowt[:], 0.0)
                nc.sync.dma_start(
                    out=outd[bass.ds(t * SUP, SUP), :]
                        .rearrange("(j p) c -> p j c", p=128),
                    in_=rowt[:])

            with tc.For_i(0, T, 1) as t:
                body(t)

        def run_pipeline():
            # ---- conv1: gather x, GEMM W1, emit raw h1 rows ----
            with nc.named_scope("passA1"):
                conv_passA(idx1, xt16, 0.0, w1s, None, sacc1, tail_conv1)
            with nc.named_scope("allgather"):
                nc.gpsimd.collective_compute(
                    "AllGather", ALU.bypass, replica_groups=groups,
                    ins=[h1loc[:]], outs=[h1ag[0:tbl2, :]],
                )
            with nc.named_scope("bn1"):
                bn_finalize(sacc1, st1i, st1o, gb1t, ab1)
                nc.sync.dma_start(out=ab1d[:], in_=ab1[:])
                for q in range(4):
                    nc.sync.dma_start(out=ab1f[q * C:(q + 1) * C, :], in_=ab1d[:])
            # ---- conv2: gather raw h1, fused bn1+relu, GEMM W2 ----
            with nc.named_scope("passA2"):
                conv_passA(idx2, h1ag, NEG, w2s, ab1f, sacc2, tail_conv2)
            with nc.named_scope("bn2"):
                bn_finalize(sacc2, st2i, st2o, gb2t, ab2)
            # ---- bn2 + residual + relu ----
            with nc.named_scope("passB2"):
                conv_passB2()

        for _rep in range(reps):
            run_pipeline()

        if debug_taps:
            taps = dict(h1loc=h1loc, scr2=scr2, st1o=st1o, st2o=st2o)
            for nm, tt in taps.items():
                d = nc.declare_dram_parameter(
                    f"dbg_{nm}", list(tt.shape),
                    F16 if nm in ("h1loc", "scr2") else F32, isOutput=True)
                nc.gpsimd.dma_start(out=d[:], in_=tt[:])
            dab = nc.declare_dram_parameter("dbg_ab1f", [128, 2], F32,
                                            isOutput=True)
            nc.gpsimd.dma_start(out=dab[:], in_=ab1f[:])

    nc.finalize()
    return nc


def host_prepare(x, nbr1, mask1, nbr2, mask2, W1, W2,
                 gamma1, beta1, gamma2, beta2,
                 rows_pc, R, T, n_cores=N_CORES):
    """Translate indices / pad / pack per-core input maps."""
    n = x.shape[0]
    assert gamma1.min() > 0, "fused bn1 relu trick needs gamma1 > 0"

    iv1 = np.where(mask1 != 0, nbr1, n).astype(np.int32)
    c_of = (nbr2.astype(np.int64)) // rows_pc
    pos2 = c_of * R + (nbr2.astype(np.int64) - c_of * rows_pc)
    iv2 = np.where(mask2 != 0, pos2, N_CORES * R).astype(np.int32)

    xt16 = np.zeros((n + 1, C), np.float16)
    xt16[:n] = x.astype(np.float16)

    def wcat(W):
        w = np.zeros((KP * C, C), np.float16)
        w[:K * C] = W.reshape(K * C, C)
        return w

    w1cat, w2cat = wcat(W1), wcat(W2)
    gb1 = np.stack([gamma1, beta1]).astype(np.float32)
    gb2 = np.stack([gamma2, beta2]).astype(np.float32)

    def pack_idx(iv, zfill, c):
        a = iv[c * rows_pc:(c + 1) * rows_pc]        # [rows_pc, 27]
        ap = np.full((R, KP), zfill, np.int32)
        ap[:rows_pc, :K] = a
        # row (t, j, p) = t*512 + j*128 + p ; columns (j, k)
        ap = ap.reshape(T, 4, 128, KP).transpose(0, 2, 1, 3).reshape(T, 128, 4 * KP)
        return np.ascontiguousarray(ap)

    in_maps = []
    for c in range(n_cores):
        xr = np.zeros((R, C), np.float32)
        xr[:rows_pc] = x[c * rows_pc:(c + 1) * rows_pc]
        in_maps.append(dict(
            xt16=xt16,
            xres=xr,
            idx1=pack_idx(iv1, n, c),
            idx2=pack_idx(iv2, N_CORES * R, c),
            w1cat=w1cat, w2cat=w2cat, gb1=gb1, gb2=gb2,
        ))
    return in_maps


_CACHE = {}


def _get_program():
    key = (R_FULL, T_FULL)
    if key not in _CACHE:
        _CACHE[key] = build_program(R_FULL, T_FULL, N_FULL)
    return _CACHE[key]


def kernel(x, nbr1, mask1, nbr2, mask2, W1, W2, gamma1, beta1, gamma2, beta2,
           _trace=False, _trace_kwargs=None):
    x = np.asarray(x, np.float32)
    in_maps = host_prepare(
        x, np.asarray(nbr1), np.asarray(mask1), np.asarray(nbr2),
        np.asarray(mask2), np.asarray(W1, np.float32), np.asarray(W2, np.float32),
        np.asarray(gamma1, np.float32), np.asarray(beta1, np.float32),
        np.asarray(gamma2, np.float32), np.asarray(beta2, np.float32),
        ROWS_PC, R_FULL, T_FULL)
    nc = _get_program()
    res = run_bass_kernel_spmd(nc, in_maps, list(range(N_CORES)),
                               trace=_trace, **(_trace_kwargs or {}))
    out = np.concatenate(
        [res.results[c]["out"][:ROWS_PC] for c in range(N_CORES)], axis=0)
    if _trace:
        return out, res
    return out
